# revision 17
# baseline (speedup 1.0000x reference)
"""PASA group-softmax high-pass downsample kernel for 8 Trainium2 NeuronCores.

Reference computation (n=4, c=64, h=w=128, G=2 groups, K=3, stride 2):
  xp     = reflect_pad(x, 1)
  sigma  = conv3x3(xp, conv_w)                    # [n, 18, h, w]
  sigma  = sigma * bn_scale + bn_shift            # BN (inference)
  sigma  = softmax(sigma, axis=1)                 # over all 18 channels
  sigma  = onehot(center) - sigma                 # high-pass
  out[n,g,c,i,j] = sum_k patches[n,g,c,k,i,j] * sigma[n,g,k,i,j]
  return out[:, :, ::2, ::2]                      # [4, 64, 64, 64]

Key optimizations:
  - Only stride-2 output positions are computed (4x less conv/softmax/apply
    work than the reference, which computes all positions then subsamples).
  - BN is folded into the conv weights (host-side) + exp bias (on ACT).
  - softmax division is folded to the end: out = x_c - (sum_k xp_k*E_k) * r
    with r = 1/sum(E), so the per-tap multiplier is just E = exp(sigma_bn).
  - Sharding: core = (image n, h-half). Each core's region is further split
    into two sub-halves (A/B) stacked on SBUF partitions 64..127, so every
    vector op runs with all 128 lanes active.

Per-core device layout:
  x slab  [128 part, 33 rows, 130 cols]: part p<64 -> channel p, sub-half A
          (padded rows r0..r0+32); p>=64 -> channel p-64, sub-half B
          (padded rows r0+32..r0+64). Host prepares this (reflect pad+halo).
  conv    -> PSUM sigma [128, 512]: col-group q=(half, chunk) holds channels
          at partitions 32q..32q+32 (rows 18..31 zero), 512 positions each
          (chunk = 8 output rows x 64 cols).
  exp     -> E [128, 512] in SBUF (ACT, bias = BN shift per partition).
  D       -> ones-selector matmul -> [4, 512]; r = 1/D on DVE.
  Ebig_k  -> DMA row-broadcast of E rows into channel layout [128, 16, 64].
  apply   -> DVE: acc += patch_k * Ebig_k  (9 taps); out = x_c - acc*rbig.
"""

import os
import ml_dtypes
import numpy as np

import concourse.bass as bass
import concourse.tile as tile
from concourse import bacc, mybir
from concourse.bass_utils import run_bass_kernel_spmd

F32 = mybir.dt.float32
BF16 = mybir.dt.bfloat16

N, C, H, W = 4, 64, 128, 128
G, K = 2, 3
K2 = K * K
EPS = 1e-5
NCORES = 8
HO, WO = H // 2, W // 2            # 64, 64 output spatial
ROWS_PER_CORE = HO // 2            # 32 output rows per core (half image)
ROWS_SUB = ROWS_PER_CORE // 2      # 16 output rows per sub-half (A/B)
SLAB_R, SLAB_C = 2 * ROWS_SUB + 1, W + 2   # 33 x 130 padded slab per sub-half
POS_SUB = ROWS_SUB * WO            # 1024 positions per sub-half
CHUNK_ROWS = ROWS_SUB // 2         # 8 output rows per psum chunk
CHUNK = CHUNK_ROWS * WO            # 512 positions per chunk

_compiled = None


def _build_program():
    """Build the single SPMD Bass program (same for all 8 cores)."""
    nc = bacc.Bacc(
        "TRN2", target_bir_lowering=False, debug=False, num_devices=NCORES
    )

    xab = nc.dram_tensor("xab", [128, SLAB_R, SLAB_C], F32, kind="ExternalInput")
    wts = nc.dram_tensor("wts", [128, K2, 32], F32, kind="ExternalInput")
    bias = nc.dram_tensor("bias", [128, 1], F32, kind="ExternalInput")
    sel = nc.dram_tensor("sel", [128, 4], BF16, kind="ExternalInput")
    esel = nc.dram_tensor("esel", [128, 2 * K2, 128], BF16, kind="ExternalInput")
    rsel = nc.dram_tensor("rsel", [4, 128], F32, kind="ExternalInput")
    ident = nc.dram_tensor("ident", [128, 128], BF16, kind="ExternalInput")
    y = nc.dram_tensor("y", [128, ROWS_SUB, WO], F32, kind="ExternalOutput")
    warm_out = nc.dram_tensor("warm_out", [1, 2], F32, kind="ExternalOutput")

    with tile.TileContext(nc) as tc:
        with (
            tc.tile_pool(name="singles", bufs=1) as singles,
            tc.tile_pool(name="psum", bufs=1, space="PSUM") as psum,
            tc.tile_pool(name="ebig", bufs=2, space="PSUM") as ebig_pool,
            tc.tile_pool(name="work", bufs=3) as work,
        ):
            # ---- small loads first (sync ring), bulk x on gpsimd+scalar ----
            ident_sb = singles.tile([128, 128], BF16)
            nc.sync.dma_start(ident_sb[:], ident.ap())
            w_sb = singles.tile([128, K2, 32], F32)
            nc.sync.dma_start(w_sb[:], wts.ap())
            bias_sb = singles.tile([128, 1], F32)
            nc.sync.dma_start(bias_sb[:], bias.ap())
            sel_sb = singles.tile([128, 4], BF16)
            nc.sync.dma_start(sel_sb[:], sel.ap())
            rsel_sb = singles.tile([4, 128], F32)
            nc.sync.dma_start(rsel_sb[:], rsel.ap())
            esel_sb = singles.tile([128, 2 * K2, 128], BF16)
            nc.sync.dma_start(esel_sb[:], esel.ap())

            # prewarm ACT's exp table with a dep-free activate so the
            # table load doesn't land on the critical path (and doesn't
            # block the scalar DMA ring)
            warm_in = work.tile([1, 1], F32, tag="warm_in")
            nc.gpsimd.memset(warm_in[:], 0.25)
            warm_e = work.tile([1, 1], F32, tag="warm")
            nc.scalar.activation(warm_e[:], warm_in[:],
                                 mybir.ActivationFunctionType.Exp)

            # x slab: 4 row-chunk DMAs across two rings; A chunks land first
            x_sb = singles.tile([128, SLAB_R, SLAB_C], F32)
            for h in range(2):
                p0 = 64 * h
                for ch in range(2):
                    r0 = 0 if ch == 0 else 2 * CHUNK_ROWS + 1
                    r1 = 2 * CHUNK_ROWS + 1 if ch == 0 else SLAB_R
                    eng = nc.gpsimd if ch == 0 else nc.scalar
                    eng.dma_start(
                        x_sb[p0 : p0 + 64, r0:r1],
                        xab.ap()[p0 : p0 + 64, r0:r1],
                    )

            # PE warm-up: junk matmuls on already-loaded constants keep the
            # HAM activity window busy during the x load so the conv runs
            # at 2.4 GHz from its first tap.  The accumulation chain plus the
            # warm_out reader keeps DCE from dropping them.
            warm_ps = psum.tile([128, 128], F32, tag="dr",
                                 padded_shape=[128, CHUNK])
            NWARM = 150
            for i in range(NWARM):
                nc.tensor.matmul(warm_ps[:], ident_sb[:], ident_sb[:],
                                 start=(i == 0), stop=(i == NWARM - 1),
                                 skip_group_check=True)
            warm_sb = work.tile([1, 2], F32, tag="warm_sb")
            nc.vector.tensor_copy(warm_sb[:], warm_ps[0:1, 0:2])
            nc.sync.dma_start(warm_out.ap(), warm_sb[:])

            # ---- conv: 9 taps x 4 col-groups into one PSUM bank ----
            sigma_ps = psum.tile([128, CHUNK], F32, tag="acc",
                                 padded_shape=[128, POS_SUB])
            for k in range(K2):
                dy, dx = k // K, k % K
                for q in range(4):
                    h, ch = q // 2, q % 2
                    p0 = 64 * h
                    r0 = 2 * CHUNK_ROWS * ch + dy
                    rhs = x_sb[
                        p0 : p0 + 64,
                        r0 : r0 + 2 * (CHUNK_ROWS - 1) + 1 : 2,
                        dx : dx + 2 * (WO - 1) + 1 : 2,
                    ]
                    nc.tensor.matmul(
                        sigma_ps[32 * q : 32 * q + 32, :],
                        w_sb[p0 : p0 + 64, k, :],
                        rhs,
                        start=(k == 0),
                        stop=(k == K2 - 1),
                        tile_position=(p0, 32 * q),
                        skip_group_check=True,
                    )

            # ---- E = exp(sigma + bn_shift), in bf16 ----
            e_sb = singles.tile([128, CHUNK], BF16)
            nc.scalar.activation(
                e_sb[:], sigma_ps[:], mybir.ActivationFunctionType.Exp,
                bias=bias_sb[:], scale=1.0,
            )

            # ---- denominator, reciprocal, normalized weights F = E/D ----
            d_ps = psum.tile([4, CHUNK], F32, tag="dr")
            nc.tensor.matmul(d_ps[:], sel_sb[:], e_sb[:])
            r_sb = singles.tile([4, CHUNK], F32)
            r_scr = singles.tile([4, CHUNK], F32)
            nc.vector.reciprocal_approx_accurate(r_sb[:], d_ps[:], r_scr[:])
            rbig18_ps = psum.tile([128, CHUNK], F32, tag="dr")
            nc.tensor.matmul(rbig18_ps[:], rsel_sb[:], r_sb[:])
            f_sb = singles.tile([128, CHUNK], BF16)
            nc.vector.tensor_mul(f_sb[:], e_sb[:], rbig18_ps[:])

            # ---- apply: acc_ps = sum_k patch_k * Fbig_k (adds on PE) ----
            acc_ps = psum.tile([128, POS_SUB], F32, tag="acc")
            prods = []
            for k in range(K2):
                dy, dx = k // K, k % K
                ebig = ebig_pool.tile([128, POS_SUB], F32, name=f"ebig{k}",
                                      tag="ebig")
                for ch in range(2):
                    nc.tensor.matmul(
                        ebig[:, CHUNK * ch : CHUNK * (ch + 1)],
                        esel_sb[:, 2 * k + ch, :],
                        f_sb[:],
                    )
                patch = x_sb[:, dy : dy + 2 * (ROWS_SUB - 1) + 1 : 2,
                             dx : dx + 2 * (WO - 1) + 1 : 2]
                prod = work.tile([128, POS_SUB], BF16, name=f"prod{k}",
                                 tag="prod")
                nc.vector.tensor_mul(
                    prod[:].rearrange("p (r c) -> p r c", r=ROWS_SUB),
                    patch, ebig[:].rearrange("p (r c) -> p r c", r=ROWS_SUB),
                )
                prods.append(prod)
                if k >= 1:
                    pprev = prods[k - 1]
                    for ch in range(2):
                        nc.tensor.matmul(
                            acc_ps[:, CHUNK * ch : CHUNK * (ch + 1)],
                            ident_sb[:],
                            pprev[:, CHUNK * ch : CHUNK * (ch + 1)],
                            start=(k == 1),
                            stop=False,
                            skip_group_check=True,
                        )
            for ch in range(2):
                nc.tensor.matmul(
                    acc_ps[:, CHUNK * ch : CHUNK * (ch + 1)],
                    ident_sb[:],
                    prods[K2 - 1][:, CHUNK * ch : CHUNK * (ch + 1)],
                    start=False,
                    stop=True,
                    skip_group_check=True,
                )

            # ---- combine: y = x_center - acc (4 chunks, stores overlap) ----
            y_sb = work.tile([128, ROWS_SUB, WO], F32)
            acc3 = acc_ps[:].rearrange("p (r c) -> p r c", r=ROWS_SUB)
            QR = CHUNK_ROWS // 2
            for ch in range(4):
                rr = slice(QR * ch, QR * (ch + 1))
                xc = x_sb[:, 2 * QR * ch + 1
                          : 2 * QR * ch + 2 * (QR - 1) + 2 : 2,
                          1 : 2 * (WO - 1) + 2 : 2]
                nc.vector.tensor_sub(y_sb[:, rr], xc, acc3[:, rr])
                eng = nc.sync if ch % 2 == 0 else nc.scalar
                eng.dma_start(y.ap()[:, rr], y_sb[:, rr])

    nc.compile()
    return nc


def _host_inputs(x, conv_w, gamma, beta, running_mean, running_var):
    """Prepare per-core input dicts (sharding + BN folding + reflect pad)."""
    scale = gamma / np.sqrt(running_var + EPS)
    shift = beta - running_mean * scale

    # weights: lhsT layout [tap, c, o] scaled by BN, padded to 32 outs, dup'd
    w_scaled = conv_w * scale[:, None, None, None]           # [18, 64, 3, 3]
    wl = np.transpose(w_scaled, (2, 3, 1, 0)).reshape(K2, C, G * K2)
    wl32 = np.zeros((K2, C, 32), np.float32)
    wl32[:, :, : G * K2] = wl
    wts = np.ascontiguousarray(
        np.concatenate([wl32, wl32], axis=1).transpose(1, 0, 2), np.float32
    )
    # -> [128, 9, 32]

    bias = np.zeros((128, 1), np.float32)
    for q in range(4):
        bias[32 * q : 32 * q + G * K2, 0] = shift

    sel = np.zeros((128, 4), np.float32)
    for q in range(4):
        sel[32 * q : 32 * q + G * K2, q] = 1.0
    sel = sel.astype(ml_dtypes.bfloat16)

    # esel[:, 2k+c, :]: lhsT mapping F rows -> channel-layout partitions for
    # tap k, chunk c.
    esel = np.zeros((128, 2 * K2, 128), np.float32)
    for k in range(K2):
        for c in range(2):
            for j in range(128):
                h, g = j // 64, (j % 64) // 32
                esel[32 * (2 * h + c) + g * K2 + k, 2 * k + c, j] = 1.0
    esel = esel.astype(ml_dtypes.bfloat16)

    # rsel: broadcast r rows q -> compact-layout partitions (32q..32q+32)
    rsel = np.zeros((4, 128), np.float32)
    for p in range(128):
        rsel[p // 32, p] = 1.0

    ident = np.eye(128, dtype=np.float32).astype(ml_dtypes.bfloat16)

    xpad = np.pad(x, ((0, 0), (0, 0), (1, 1), (1, 1)), mode="reflect")

    in_maps = []
    for core in range(NCORES):
        n, h = core // 2, core % 2
        r0 = 64 * h
        slab_a = xpad[n, :, r0 : r0 + SLAB_R, :]
        slab_b = xpad[n, :, r0 + SLAB_R - 1 : r0 + 2 * SLAB_R - 1, :]
        xab = np.ascontiguousarray(
            np.concatenate([slab_a, slab_b], axis=0), np.float32
        )
        in_maps.append(
            {"xab": xab, "wts": wts, "bias": bias, "sel": sel,
             "esel": esel, "rsel": rsel, "ident": ident}
        )
    return in_maps


def _gather_output(results):
    out = np.empty((N, C, HO, WO), np.float32)
    for core, res in enumerate(results):
        n, h = core // 2, core % 2
        ycore = res["y"].reshape(2, C, ROWS_SUB, WO)
        out[n, :, 32 * h : 32 * h + ROWS_SUB, :] = ycore[0]
        out[n, :, 32 * h + ROWS_SUB : 32 * h + 2 * ROWS_SUB, :] = ycore[1]
    return out


def _ensure_ntff_hook():
    """Install the axon NTFF profile hook if the image's antenv lacks it."""
    try:
        from antenv import axon_hooks  # noqa: F401
        return
    except ImportError:
        pass
    try:
        import sys
        import types

        import antenv
        from trn_agent_boot.trn_boot import _ntff_profile_via_ctypes

        hook = _ntff_profile_via_ctypes("/opt/axon/libaxon_pjrt.so")
        mod = types.ModuleType("antenv.axon_hooks")
        state = {"hook": hook}
        mod.get_axon_ntff_profile_hook = lambda: state["hook"]
        mod.set_axon_ntff_profile_hook = lambda h: state.update(hook=h)
        sys.modules["antenv.axon_hooks"] = mod
        antenv.axon_hooks = mod
    except Exception:
        pass


def kernel(x, conv_w, gamma, beta, running_mean, running_var):
    global _compiled
    x = np.asarray(x, np.float32)
    conv_w = np.asarray(conv_w, np.float32)
    gamma = np.asarray(gamma, np.float32)
    beta = np.asarray(beta, np.float32)
    running_mean = np.asarray(running_mean, np.float32)
    running_var = np.asarray(running_var, np.float32)

    if _compiled is None:
        _compiled = _build_program()
    nc = _compiled

    in_maps = _host_inputs(x, conv_w, gamma, beta, running_mean, running_var)
    trace = bool(int(os.environ.get("PASA_TRACE", "0")))
    if trace:
        _ensure_ntff_hook()
    res = run_bass_kernel_spmd(
        nc, in_maps, core_ids=list(range(NCORES)), trace=trace
    )
    kernel.last_results = res
    return _gather_output(res.results)


if __name__ == "__main__":
    # quick CoreSim check of core 0 against a numpy re-implementation
    from concourse.bass_interp import CoreSim

    rng = np.random.default_rng(0)
    x = rng.standard_normal((N, C, H, W), np.float32)
    conv_w = (rng.standard_normal((G * K2, C, K, K), np.float32)
              * np.sqrt(2.0 / (G * K2 * K * K)))
    gamma = rng.uniform(0.5, 1.5, G * K2).astype(np.float32)
    beta = (rng.standard_normal(G * K2) * 0.1).astype(np.float32)
    rmean = (rng.standard_normal(G * K2) * 0.1).astype(np.float32)
    rvar = rng.uniform(0.5, 1.5, G * K2).astype(np.float32)

    nc = _build_program()
    in_maps = _host_inputs(x, conv_w, gamma, beta, rmean, rvar)
    sim = CoreSim(nc)
    for k, v in in_maps[0].items():
        sim.tensor(k)[:] = v
    sim.simulate(check_with_hw=False)
    ysim = np.array(sim.tensor("y")).reshape(2, C, ROWS_SUB, WO)

    # numpy reference for core 0 region (image 0, output rows 0..32)
    scale = gamma / np.sqrt(rvar + EPS)
    shift = beta - rmean * scale
    xpad = np.pad(x[0], ((0, 0), (1, 1), (1, 1)), mode="reflect")
    sig = np.zeros((G * K2, 32, WO), np.float32)
    for o in range(G * K2):
        for dy in range(K):
            for dx in range(K):
                sig[o] += np.einsum(
                    "crw->rw",
                    conv_w[o, :, dy, dx][:, None, None]
                    * xpad[:, dy : dy + 64 : 2, dx : dx + 128 : 2],
                )
    sig = sig * scale[:, None, None] + shift[:, None, None]
    e = np.exp(sig)
    r = 1.0 / e.sum(0)
    acc = np.zeros((C, 32, WO), np.float32)
    for g in range(G):
        for k in range(K2):
            dy, dx = k // K, k % K
            acc[32 * g : 32 * g + 32] += (
                xpad[32 * g : 32 * g + 32, dy : dy + 64 : 2, dx : dx + 128 : 2]
                * e[g * K2 + k][None]
            )
    ref = (xpad[:, 1:65:2, 1:129:2] - acc * r[None]).astype(np.float32)

    got = np.concatenate([ysim[0], ysim[1]], axis=1)
    err = np.abs(got - ref).max() / np.abs(ref).max()
    print("sim rel err:", err)


# revision 18
# speedup vs baseline: 1.0484x; 1.0484x over previous
"""PASA group-softmax high-pass downsample kernel for 8 Trainium2 NeuronCores.

Reference computation (n=4, c=64, h=w=128, G=2 groups, K=3, stride 2):
  xp     = reflect_pad(x, 1)
  sigma  = conv3x3(xp, conv_w)                    # [n, 18, h, w]
  sigma  = sigma * bn_scale + bn_shift            # BN (inference)
  sigma  = softmax(sigma, axis=1)                 # over all 18 channels
  sigma  = onehot(center) - sigma                 # high-pass
  out[n,g,c,i,j] = sum_k patches[n,g,c,k,i,j] * sigma[n,g,k,i,j]
  return out[:, :, ::2, ::2]                      # [4, 64, 64, 64]

Key optimizations:
  - Only stride-2 output positions are computed (4x less conv/softmax/apply
    work than the reference, which computes all positions then subsamples).
  - BN is folded into the conv weights (host-side) + exp bias (on ACT).
  - softmax division is folded to the end: out = x_c - (sum_k xp_k*E_k) * r
    with r = 1/sum(E), so the per-tap multiplier is just E = exp(sigma_bn).
  - Sharding: core = (image n, h-half). Each core's region is further split
    into two sub-halves (A/B) stacked on SBUF partitions 64..127, so every
    vector op runs with all 128 lanes active.

Per-core device layout:
  x slab  [128 part, 33 rows, 130 cols]: part p<64 -> channel p, sub-half A
          (padded rows r0..r0+32); p>=64 -> channel p-64, sub-half B
          (padded rows r0+32..r0+64). Host prepares this (reflect pad+halo).
  conv    -> PSUM sigma [128, 512]: col-group q=(half, chunk) holds channels
          at partitions 32q..32q+32 (rows 18..31 zero), 512 positions each
          (chunk = 8 output rows x 64 cols).
  exp     -> E [128, 512] in SBUF (ACT, bias = BN shift per partition).
  D       -> ones-selector matmul -> [4, 512]; r = 1/D on DVE.
  Ebig_k  -> DMA row-broadcast of E rows into channel layout [128, 16, 64].
  apply   -> DVE: acc += patch_k * Ebig_k  (9 taps); out = x_c - acc*rbig.
"""

import os
import ml_dtypes
import numpy as np

import concourse.bass as bass
import concourse.tile as tile
from concourse import bacc, mybir
from concourse.bass_utils import run_bass_kernel_spmd

F32 = mybir.dt.float32
BF16 = mybir.dt.bfloat16

N, C, H, W = 4, 64, 128, 128
G, K = 2, 3
K2 = K * K
EPS = 1e-5
NCORES = 8
HO, WO = H // 2, W // 2            # 64, 64 output spatial
ROWS_PER_CORE = HO // 2            # 32 output rows per core (half image)
ROWS_SUB = ROWS_PER_CORE // 2      # 16 output rows per sub-half (A/B)
SLAB_R, SLAB_C = 2 * ROWS_SUB + 1, W + 2   # 33 x 130 padded slab per sub-half
POS_SUB = ROWS_SUB * WO            # 1024 positions per sub-half
CHUNK_ROWS = ROWS_SUB // 2         # 8 output rows per psum chunk
CHUNK = CHUNK_ROWS * WO            # 512 positions per chunk

_compiled = None


def _build_program():
    """Build the single SPMD Bass program (same for all 8 cores)."""
    nc = bacc.Bacc(
        "TRN2", target_bir_lowering=False, debug=False, num_devices=NCORES
    )

    xab = nc.dram_tensor("xab", [128, SLAB_R, SLAB_C], F32, kind="ExternalInput")
    wts = nc.dram_tensor("wts", [128, K2, 32], F32, kind="ExternalInput")
    bias = nc.dram_tensor("bias", [128, 1], F32, kind="ExternalInput")
    sel = nc.dram_tensor("sel", [128, 4], BF16, kind="ExternalInput")
    esel = nc.dram_tensor("esel", [128, 2 * K2, 128], BF16, kind="ExternalInput")
    rsel = nc.dram_tensor("rsel", [4, 128], F32, kind="ExternalInput")
    ident = nc.dram_tensor("ident", [128, 128], BF16, kind="ExternalInput")
    y = nc.dram_tensor("y", [128, ROWS_SUB, WO], F32, kind="ExternalOutput")
    warm_out = nc.dram_tensor("warm_out", [1, 2], F32, kind="ExternalOutput")

    with tile.TileContext(nc) as tc:
        with (
            tc.tile_pool(name="singles", bufs=1) as singles,
            tc.tile_pool(name="psum", bufs=1, space="PSUM") as psum,
            tc.tile_pool(name="ebig", bufs=2, space="PSUM") as ebig_pool,
            tc.tile_pool(name="work", bufs=3) as work,
        ):
            # ---- small loads first (sync ring), bulk x on gpsimd+scalar ----
            ident_sb = singles.tile([128, 128], BF16)
            nc.sync.dma_start(ident_sb[:], ident.ap())
            w_sb = singles.tile([128, K2, 32], F32)
            nc.sync.dma_start(w_sb[:], wts.ap())
            bias_sb = singles.tile([128, 1], F32)
            nc.sync.dma_start(bias_sb[:], bias.ap())
            sel_sb = singles.tile([128, 4], BF16)
            nc.sync.dma_start(sel_sb[:], sel.ap())
            rsel_sb = singles.tile([4, 128], F32)
            nc.sync.dma_start(rsel_sb[:], rsel.ap())
            esel_sb = singles.tile([128, 2 * K2, 128], BF16)
            nc.sync.dma_start(esel_sb[:], esel.ap())

            # prewarm ACT's exp table with a dep-free activate so the
            # table load doesn't land on the critical path (and doesn't
            # block the scalar DMA ring)
            warm_in = work.tile([1, 1], F32, tag="warm_in")
            nc.gpsimd.memset(warm_in[:], 0.25)
            warm_e = work.tile([1, 1], F32, tag="warm")
            nc.scalar.activation(warm_e[:], warm_in[:],
                                 mybir.ActivationFunctionType.Exp)

            # x slab: 4 row-chunk DMAs across two rings; A chunks land first
            x_sb = singles.tile([128, SLAB_R, SLAB_C], F32)
            for h in range(2):
                p0 = 64 * h
                for ch in range(2):
                    r0 = 0 if ch == 0 else 2 * CHUNK_ROWS + 1
                    r1 = 2 * CHUNK_ROWS + 1 if ch == 0 else SLAB_R
                    eng = nc.gpsimd if ch == 0 else nc.scalar
                    eng.dma_start(
                        x_sb[p0 : p0 + 64, r0:r1],
                        xab.ap()[p0 : p0 + 64, r0:r1],
                    )

            # PE warm-up: junk matmuls on already-loaded constants keep the
            # HAM activity window busy during the x load so the conv runs
            # at 2.4 GHz from its first tap.  The accumulation chain plus the
            # warm_out reader keeps DCE from dropping them.
            warm_ps = psum.tile([128, 128], F32, tag="dr",
                                 padded_shape=[128, CHUNK])
            NWARM = 160
            for i in range(NWARM):
                nc.tensor.matmul(warm_ps[:], ident_sb[:], ident_sb[:],
                                 start=(i == 0), stop=(i == NWARM - 1),
                                 skip_group_check=True)
            warm_sb = work.tile([1, 2], F32, tag="warm_sb")
            nc.vector.tensor_copy(warm_sb[:], warm_ps[0:1, 0:2])
            nc.sync.dma_start(warm_out.ap(), warm_sb[:])

            # ---- conv: 9 taps x 4 col-groups into one PSUM bank ----
            sigma_ps = psum.tile([128, CHUNK], F32, tag="acc",
                                 padded_shape=[128, POS_SUB])
            for k in range(K2):
                dy, dx = k // K, k % K
                for q in range(4):
                    h, ch = q // 2, q % 2
                    p0 = 64 * h
                    r0 = 2 * CHUNK_ROWS * ch + dy
                    rhs = x_sb[
                        p0 : p0 + 64,
                        r0 : r0 + 2 * (CHUNK_ROWS - 1) + 1 : 2,
                        dx : dx + 2 * (WO - 1) + 1 : 2,
                    ]
                    nc.tensor.matmul(
                        sigma_ps[32 * q : 32 * q + 32, :],
                        w_sb[p0 : p0 + 64, k, :],
                        rhs,
                        start=(k == 0),
                        stop=(k == K2 - 1),
                        tile_position=(p0, 32 * q),
                        skip_group_check=True,
                    )

            # ---- E = exp(sigma + bn_shift), in bf16 ----
            e_sb = singles.tile([128, CHUNK], BF16)
            nc.scalar.activation(
                e_sb[:], sigma_ps[:], mybir.ActivationFunctionType.Exp,
                bias=bias_sb[:], scale=1.0,
            )

            # ---- denominator, reciprocal, normalized weights F = E/D ----
            d_ps = psum.tile([4, CHUNK], F32, tag="dr")
            nc.tensor.matmul(d_ps[:], sel_sb[:], e_sb[:])
            r_sb = singles.tile([4, CHUNK], F32)
            r_scr = singles.tile([4, CHUNK], F32)
            nc.vector.reciprocal_approx_accurate(r_sb[:], d_ps[:], r_scr[:])
            rbig18_ps = psum.tile([128, CHUNK], F32, tag="dr")
            nc.tensor.matmul(rbig18_ps[:], rsel_sb[:], r_sb[:])
            f_sb = singles.tile([128, CHUNK], BF16)
            nc.vector.tensor_mul(f_sb[:], e_sb[:], rbig18_ps[:])

            # ---- apply: acc_ps = sum_k patch_k * Fbig_k (adds on PE) ----
            acc_ps = psum.tile([128, POS_SUB], F32, tag="acc")
            prods = []
            for k in range(K2):
                dy, dx = k // K, k % K
                ebig = ebig_pool.tile([128, POS_SUB], F32, name=f"ebig{k}",
                                      tag="ebig")
                for ch in range(2):
                    nc.tensor.matmul(
                        ebig[:, CHUNK * ch : CHUNK * (ch + 1)],
                        esel_sb[:, 2 * k + ch, :],
                        f_sb[:],
                    )
                patch = x_sb[:, dy : dy + 2 * (ROWS_SUB - 1) + 1 : 2,
                             dx : dx + 2 * (WO - 1) + 1 : 2]
                prod = work.tile([128, POS_SUB], BF16, name=f"prod{k}",
                                 tag="prod")
                nc.vector.tensor_mul(
                    prod[:].rearrange("p (r c) -> p r c", r=ROWS_SUB),
                    patch, ebig[:].rearrange("p (r c) -> p r c", r=ROWS_SUB),
                )
                prods.append(prod)
                if k >= 1:
                    pprev = prods[k - 1]
                    for ch in range(2):
                        nc.tensor.matmul(
                            acc_ps[:, CHUNK * ch : CHUNK * (ch + 1)],
                            ident_sb[:],
                            pprev[:, CHUNK * ch : CHUNK * (ch + 1)],
                            start=(k == 1),
                            stop=False,
                            skip_group_check=True,
                        )
            for ch in range(2):
                nc.tensor.matmul(
                    acc_ps[:, CHUNK * ch : CHUNK * (ch + 1)],
                    ident_sb[:],
                    prods[K2 - 1][:, CHUNK * ch : CHUNK * (ch + 1)],
                    start=False,
                    stop=True,
                    skip_group_check=True,
                )

            # ---- combine: y = x_center - acc (2 chunks, stores overlap) ----
            y_sb = work.tile([128, ROWS_SUB, WO], F32)
            acc3 = acc_ps[:].rearrange("p (r c) -> p r c", r=ROWS_SUB)
            for ch in range(2):
                rr = slice(CHUNK_ROWS * ch, CHUNK_ROWS * (ch + 1))
                xc = x_sb[:, 2 * CHUNK_ROWS * ch + 1
                          : 2 * CHUNK_ROWS * ch + 2 * (CHUNK_ROWS - 1) + 2 : 2,
                          1 : 2 * (WO - 1) + 2 : 2]
                nc.vector.tensor_sub(y_sb[:, rr], xc, acc3[:, rr])
                eng = nc.sync if ch == 0 else nc.scalar
                eng.dma_start(y.ap()[:, rr], y_sb[:, rr])

    nc.compile()
    return nc


def _host_inputs(x, conv_w, gamma, beta, running_mean, running_var):
    """Prepare per-core input dicts (sharding + BN folding + reflect pad)."""
    scale = gamma / np.sqrt(running_var + EPS)
    shift = beta - running_mean * scale

    # weights: lhsT layout [tap, c, o] scaled by BN, padded to 32 outs, dup'd
    w_scaled = conv_w * scale[:, None, None, None]           # [18, 64, 3, 3]
    wl = np.transpose(w_scaled, (2, 3, 1, 0)).reshape(K2, C, G * K2)
    wl32 = np.zeros((K2, C, 32), np.float32)
    wl32[:, :, : G * K2] = wl
    wts = np.ascontiguousarray(
        np.concatenate([wl32, wl32], axis=1).transpose(1, 0, 2), np.float32
    )
    # -> [128, 9, 32]

    bias = np.zeros((128, 1), np.float32)
    for q in range(4):
        bias[32 * q : 32 * q + G * K2, 0] = shift

    sel = np.zeros((128, 4), np.float32)
    for q in range(4):
        sel[32 * q : 32 * q + G * K2, q] = 1.0
    sel = sel.astype(ml_dtypes.bfloat16)

    # esel[:, 2k+c, :]: lhsT mapping F rows -> channel-layout partitions for
    # tap k, chunk c.
    esel = np.zeros((128, 2 * K2, 128), np.float32)
    for k in range(K2):
        for c in range(2):
            for j in range(128):
                h, g = j // 64, (j % 64) // 32
                esel[32 * (2 * h + c) + g * K2 + k, 2 * k + c, j] = 1.0
    esel = esel.astype(ml_dtypes.bfloat16)

    # rsel: broadcast r rows q -> compact-layout partitions (32q..32q+32)
    rsel = np.zeros((4, 128), np.float32)
    for p in range(128):
        rsel[p // 32, p] = 1.0

    ident = np.eye(128, dtype=np.float32).astype(ml_dtypes.bfloat16)

    xpad = np.pad(x, ((0, 0), (0, 0), (1, 1), (1, 1)), mode="reflect")

    in_maps = []
    for core in range(NCORES):
        n, h = core // 2, core % 2
        r0 = 64 * h
        slab_a = xpad[n, :, r0 : r0 + SLAB_R, :]
        slab_b = xpad[n, :, r0 + SLAB_R - 1 : r0 + 2 * SLAB_R - 1, :]
        xab = np.ascontiguousarray(
            np.concatenate([slab_a, slab_b], axis=0), np.float32
        )
        in_maps.append(
            {"xab": xab, "wts": wts, "bias": bias, "sel": sel,
             "esel": esel, "rsel": rsel, "ident": ident}
        )
    return in_maps


def _gather_output(results):
    out = np.empty((N, C, HO, WO), np.float32)
    for core, res in enumerate(results):
        n, h = core // 2, core % 2
        ycore = res["y"].reshape(2, C, ROWS_SUB, WO)
        out[n, :, 32 * h : 32 * h + ROWS_SUB, :] = ycore[0]
        out[n, :, 32 * h + ROWS_SUB : 32 * h + 2 * ROWS_SUB, :] = ycore[1]
    return out


def _ensure_ntff_hook():
    """Install the axon NTFF profile hook if the image's antenv lacks it."""
    try:
        from antenv import axon_hooks  # noqa: F401
        return
    except ImportError:
        pass
    try:
        import sys
        import types

        import antenv
        from trn_agent_boot.trn_boot import _ntff_profile_via_ctypes

        hook = _ntff_profile_via_ctypes("/opt/axon/libaxon_pjrt.so")
        mod = types.ModuleType("antenv.axon_hooks")
        state = {"hook": hook}
        mod.get_axon_ntff_profile_hook = lambda: state["hook"]
        mod.set_axon_ntff_profile_hook = lambda h: state.update(hook=h)
        sys.modules["antenv.axon_hooks"] = mod
        antenv.axon_hooks = mod
    except Exception:
        pass


def kernel(x, conv_w, gamma, beta, running_mean, running_var):
    global _compiled
    x = np.asarray(x, np.float32)
    conv_w = np.asarray(conv_w, np.float32)
    gamma = np.asarray(gamma, np.float32)
    beta = np.asarray(beta, np.float32)
    running_mean = np.asarray(running_mean, np.float32)
    running_var = np.asarray(running_var, np.float32)

    if _compiled is None:
        _compiled = _build_program()
    nc = _compiled

    in_maps = _host_inputs(x, conv_w, gamma, beta, running_mean, running_var)
    trace = bool(int(os.environ.get("PASA_TRACE", "0")))
    if trace:
        _ensure_ntff_hook()
    res = run_bass_kernel_spmd(
        nc, in_maps, core_ids=list(range(NCORES)), trace=trace
    )
    kernel.last_results = res
    return _gather_output(res.results)


if __name__ == "__main__":
    # quick CoreSim check of core 0 against a numpy re-implementation
    from concourse.bass_interp import CoreSim

    rng = np.random.default_rng(0)
    x = rng.standard_normal((N, C, H, W), np.float32)
    conv_w = (rng.standard_normal((G * K2, C, K, K), np.float32)
              * np.sqrt(2.0 / (G * K2 * K * K)))
    gamma = rng.uniform(0.5, 1.5, G * K2).astype(np.float32)
    beta = (rng.standard_normal(G * K2) * 0.1).astype(np.float32)
    rmean = (rng.standard_normal(G * K2) * 0.1).astype(np.float32)
    rvar = rng.uniform(0.5, 1.5, G * K2).astype(np.float32)

    nc = _build_program()
    in_maps = _host_inputs(x, conv_w, gamma, beta, rmean, rvar)
    sim = CoreSim(nc)
    for k, v in in_maps[0].items():
        sim.tensor(k)[:] = v
    sim.simulate(check_with_hw=False)
    ysim = np.array(sim.tensor("y")).reshape(2, C, ROWS_SUB, WO)

    # numpy reference for core 0 region (image 0, output rows 0..32)
    scale = gamma / np.sqrt(rvar + EPS)
    shift = beta - rmean * scale
    xpad = np.pad(x[0], ((0, 0), (1, 1), (1, 1)), mode="reflect")
    sig = np.zeros((G * K2, 32, WO), np.float32)
    for o in range(G * K2):
        for dy in range(K):
            for dx in range(K):
                sig[o] += np.einsum(
                    "crw->rw",
                    conv_w[o, :, dy, dx][:, None, None]
                    * xpad[:, dy : dy + 64 : 2, dx : dx + 128 : 2],
                )
    sig = sig * scale[:, None, None] + shift[:, None, None]
    e = np.exp(sig)
    r = 1.0 / e.sum(0)
    acc = np.zeros((C, 32, WO), np.float32)
    for g in range(G):
        for k in range(K2):
            dy, dx = k // K, k % K
            acc[32 * g : 32 * g + 32] += (
                xpad[32 * g : 32 * g + 32, dy : dy + 64 : 2, dx : dx + 128 : 2]
                * e[g * K2 + k][None]
            )
    ref = (xpad[:, 1:65:2, 1:129:2] - acc * r[None]).astype(np.float32)

    got = np.concatenate([ysim[0], ysim[1]], axis=1)
    err = np.abs(got - ref).max() / np.abs(ref).max()
    print("sim rel err:", err)


# revision 19
# speedup vs baseline: 1.2131x; 1.1570x over previous
"""PASA group-softmax high-pass downsample kernel for 8 Trainium2 NeuronCores.

Reference computation (n=4, c=64, h=w=128, G=2 groups, K=3, stride 2):
  xp     = reflect_pad(x, 1)
  sigma  = conv3x3(xp, conv_w)                    # [n, 18, h, w]
  sigma  = sigma * bn_scale + bn_shift            # BN (inference)
  sigma  = softmax(sigma, axis=1)                 # over all 18 channels
  sigma  = onehot(center) - sigma                 # high-pass
  out[n,g,c,i,j] = sum_k patches[n,g,c,k,i,j] * sigma[n,g,k,i,j]
  return out[:, :, ::2, ::2]                      # [4, 64, 64, 64]

Key optimizations:
  - Only stride-2 output positions are computed (4x less conv/softmax/apply
    work than the reference, which computes all positions then subsamples).
  - BN is folded into the conv weights (host-side) + exp bias (on ACT).
  - softmax division is folded to the end: out = x_c - (sum_k xp_k*E_k) * r
    with r = 1/sum(E), so the per-tap multiplier is just E = exp(sigma_bn).
  - Sharding: core = (image n, h-half). Each core's region is further split
    into two sub-halves (A/B) stacked on SBUF partitions 64..127, so every
    vector op runs with all 128 lanes active.

Per-core device layout:
  x slab  [128 part, 33 rows, 130 cols]: part p<64 -> channel p, sub-half A
          (padded rows r0..r0+32); p>=64 -> channel p-64, sub-half B
          (padded rows r0+32..r0+64). Host prepares this (reflect pad+halo).
  conv    -> PSUM sigma [128, 512]: col-group q=(half, chunk) holds channels
          at partitions 32q..32q+32 (rows 18..31 zero), 512 positions each
          (chunk = 8 output rows x 64 cols).
  exp     -> E [128, 512] in SBUF (ACT, bias = BN shift per partition).
  D       -> ones-selector matmul -> [4, 512]; r = 1/D on DVE.
  Ebig_k  -> DMA row-broadcast of E rows into channel layout [128, 16, 64].
  apply   -> DVE: acc += patch_k * Ebig_k  (9 taps); out = x_c - acc*rbig.
"""

import os
import ml_dtypes
import numpy as np

import concourse.bass as bass
import concourse.tile as tile
from concourse import bacc, mybir
from concourse.bass_utils import run_bass_kernel_spmd

F32 = mybir.dt.float32
BF16 = mybir.dt.bfloat16

N, C, H, W = 4, 64, 128, 128
G, K = 2, 3
K2 = K * K
EPS = 1e-5
NCORES = 8
HO, WO = H // 2, W // 2            # 64, 64 output spatial
ROWS_PER_CORE = HO // 2            # 32 output rows per core (half image)
ROWS_SUB = ROWS_PER_CORE // 2      # 16 output rows per sub-half (A/B)
SLAB_R, SLAB_C = 2 * ROWS_SUB + 1, W + 2   # 33 x 130 padded slab per sub-half
POS_SUB = ROWS_SUB * WO            # 1024 positions per sub-half
CHUNK_ROWS = ROWS_SUB // 2         # 8 output rows per psum chunk
CHUNK = CHUNK_ROWS * WO            # 512 positions per chunk

_compiled = None


def _build_program():
    """Build the single SPMD Bass program (same for all 8 cores)."""
    nc = bacc.Bacc(
        "TRN2", target_bir_lowering=False, debug=False, num_devices=NCORES
    )

    xab = nc.dram_tensor("xab", [128, SLAB_R, SLAB_C], BF16, kind="ExternalInput")
    xcen = nc.dram_tensor("xcen", [128, ROWS_SUB, WO], F32, kind="ExternalInput")
    wts = nc.dram_tensor("wts", [128, K2, 32], BF16, kind="ExternalInput")
    bias = nc.dram_tensor("bias", [128, 1], F32, kind="ExternalInput")
    sel = nc.dram_tensor("sel", [128, 4], BF16, kind="ExternalInput")
    esel = nc.dram_tensor("esel", [128, 2 * K2, 128], BF16, kind="ExternalInput")
    rsel = nc.dram_tensor("rsel", [4, 128], F32, kind="ExternalInput")
    ident = nc.dram_tensor("ident", [128, 128], BF16, kind="ExternalInput")
    y = nc.dram_tensor("y", [128, ROWS_SUB, WO], F32, kind="ExternalOutput")
    warm_out = nc.dram_tensor("warm_out", [1, 2], F32, kind="ExternalOutput")

    with tile.TileContext(nc) as tc:
        with (
            tc.tile_pool(name="singles", bufs=1) as singles,
            tc.tile_pool(name="psum", bufs=1, space="PSUM") as psum,
            tc.tile_pool(name="ebig", bufs=2, space="PSUM") as ebig_pool,
            tc.tile_pool(name="work", bufs=3) as work,
        ):
            # ---- small loads first (sync ring), bulk x on gpsimd+scalar ----
            ident_sb = singles.tile([128, 128], BF16)
            nc.sync.dma_start(ident_sb[:], ident.ap())
            w_sb = singles.tile([128, K2, 32], BF16)
            nc.sync.dma_start(w_sb[:], wts.ap())
            bias_sb = singles.tile([128, 1], F32)
            nc.sync.dma_start(bias_sb[:], bias.ap())
            sel_sb = singles.tile([128, 4], BF16)
            nc.sync.dma_start(sel_sb[:], sel.ap())
            rsel_sb = singles.tile([4, 128], F32)
            nc.sync.dma_start(rsel_sb[:], rsel.ap())
            esel_sb = singles.tile([128, 2 * K2, 128], BF16)
            nc.sync.dma_start(esel_sb[:], esel.ap())
            xc_sb = singles.tile([128, ROWS_SUB, WO], F32)
            nc.sync.dma_start(xc_sb[:], xcen.ap())

            # prewarm ACT's exp table with a dep-free activate so the
            # table load doesn't land on the critical path (and doesn't
            # block the scalar DMA ring)
            warm_in = work.tile([1, 1], F32, tag="warm_in")
            nc.gpsimd.memset(warm_in[:], 0.25)
            warm_e = work.tile([1, 1], F32, tag="warm")
            nc.scalar.activation(warm_e[:], warm_in[:],
                                 mybir.ActivationFunctionType.Exp)

            # x slab: 4 row-chunk DMAs across two rings; A chunks land first
            x_sb = singles.tile([128, SLAB_R, SLAB_C], BF16)
            for h in range(2):
                p0 = 64 * h
                for ch in range(2):
                    r0 = 0 if ch == 0 else 2 * CHUNK_ROWS + 1
                    r1 = 2 * CHUNK_ROWS + 1 if ch == 0 else SLAB_R
                    eng = nc.gpsimd if ch == 0 else nc.scalar
                    eng.dma_start(
                        x_sb[p0 : p0 + 64, r0:r1],
                        xab.ap()[p0 : p0 + 64, r0:r1],
                    )

            # PE warm-up: junk matmuls on already-loaded constants keep the
            # HAM activity window busy during the x load so the conv runs
            # at 2.4 GHz from its first tap.  The accumulation chain plus the
            # warm_out reader keeps DCE from dropping them.
            warm_ps = psum.tile([128, 128], F32, tag="dr",
                                 padded_shape=[128, CHUNK])
            NWARM = 160
            for i in range(NWARM):
                nc.tensor.matmul(warm_ps[:], ident_sb[:], ident_sb[:],
                                 start=(i == 0), stop=(i == NWARM - 1),
                                 skip_group_check=True)
            warm_sb = work.tile([1, 2], F32, tag="warm_sb")
            nc.vector.tensor_copy(warm_sb[:], warm_ps[0:1, 0:2])
            nc.sync.dma_start(warm_out.ap(), warm_sb[:])

            # ---- conv: 9 taps x 4 col-groups into one PSUM bank ----
            sigma_ps = psum.tile([128, CHUNK], F32, tag="acc",
                                 padded_shape=[128, POS_SUB])
            for k in range(K2):
                dy, dx = k // K, k % K
                for q in range(4):
                    h, ch = q // 2, q % 2
                    p0 = 64 * h
                    r0 = 2 * CHUNK_ROWS * ch + dy
                    rhs = x_sb[
                        p0 : p0 + 64,
                        r0 : r0 + 2 * (CHUNK_ROWS - 1) + 1 : 2,
                        dx : dx + 2 * (WO - 1) + 1 : 2,
                    ]
                    nc.tensor.matmul(
                        sigma_ps[32 * q : 32 * q + 32, :],
                        w_sb[p0 : p0 + 64, k, :],
                        rhs,
                        start=(k == 0),
                        stop=(k == K2 - 1),
                        tile_position=(p0, 32 * q),
                        skip_group_check=True,
                    )

            # ---- E = exp(sigma + bn_shift), in bf16 ----
            e_sb = singles.tile([128, CHUNK], BF16)
            nc.scalar.activation(
                e_sb[:], sigma_ps[:], mybir.ActivationFunctionType.Exp,
                bias=bias_sb[:], scale=1.0,
            )

            # ---- denominator, reciprocal, normalized weights F = E/D ----
            d_ps = psum.tile([4, CHUNK], F32, tag="dr")
            nc.tensor.matmul(d_ps[:], sel_sb[:], e_sb[:])
            r_sb = singles.tile([4, CHUNK], F32)
            r_scr = singles.tile([4, CHUNK], F32)
            nc.vector.reciprocal_approx_accurate(r_sb[:], d_ps[:], r_scr[:])
            rbig18_ps = psum.tile([128, CHUNK], F32, tag="dr")
            nc.tensor.matmul(rbig18_ps[:], rsel_sb[:], r_sb[:])
            f_sb = singles.tile([128, CHUNK], BF16)
            nc.vector.tensor_mul(f_sb[:], e_sb[:], rbig18_ps[:])

            # ---- apply: acc_ps = sum_k patch_k * Fbig_k (adds on PE) ----
            acc_ps = psum.tile([128, POS_SUB], F32, tag="acc")
            prods = []
            for k in range(K2):
                dy, dx = k // K, k % K
                ebig = ebig_pool.tile([128, POS_SUB], F32, name=f"ebig{k}",
                                      tag="ebig")
                for ch in range(2):
                    nc.tensor.matmul(
                        ebig[:, CHUNK * ch : CHUNK * (ch + 1)],
                        esel_sb[:, 2 * k + ch, :],
                        f_sb[:],
                    )
                patch = x_sb[:, dy : dy + 2 * (ROWS_SUB - 1) + 1 : 2,
                             dx : dx + 2 * (WO - 1) + 1 : 2]
                prod = work.tile([128, POS_SUB], BF16, name=f"prod{k}",
                                 tag="prod")
                nc.vector.tensor_mul(
                    prod[:].rearrange("p (r c) -> p r c", r=ROWS_SUB),
                    patch, ebig[:].rearrange("p (r c) -> p r c", r=ROWS_SUB),
                )
                prods.append(prod)
                if k >= 1:
                    pprev = prods[k - 1]
                    for ch in range(2):
                        nc.tensor.matmul(
                            acc_ps[:, CHUNK * ch : CHUNK * (ch + 1)],
                            ident_sb[:],
                            pprev[:, CHUNK * ch : CHUNK * (ch + 1)],
                            start=(k == 1),
                            stop=False,
                            skip_group_check=True,
                        )
            for ch in range(2):
                nc.tensor.matmul(
                    acc_ps[:, CHUNK * ch : CHUNK * (ch + 1)],
                    ident_sb[:],
                    prods[K2 - 1][:, CHUNK * ch : CHUNK * (ch + 1)],
                    start=False,
                    stop=True,
                    skip_group_check=True,
                )

            # ---- combine: y = x_center - acc (2 chunks, stores overlap) ----
            y_sb = work.tile([128, ROWS_SUB, WO], F32)
            acc3 = acc_ps[:].rearrange("p (r c) -> p r c", r=ROWS_SUB)
            for ch in range(2):
                rr = slice(CHUNK_ROWS * ch, CHUNK_ROWS * (ch + 1))
                nc.vector.tensor_sub(y_sb[:, rr], xc_sb[:, rr],
                                     acc3[:, rr])
                eng = nc.sync if ch == 0 else nc.scalar
                eng.dma_start(y.ap()[:, rr], y_sb[:, rr])

    nc.compile()
    return nc


def _host_inputs(x, conv_w, gamma, beta, running_mean, running_var):
    """Prepare per-core input dicts (sharding + BN folding + reflect pad)."""
    scale = gamma / np.sqrt(running_var + EPS)
    shift = beta - running_mean * scale

    # weights: lhsT layout [tap, c, o] scaled by BN, padded to 32 outs, dup'd
    w_scaled = conv_w * scale[:, None, None, None]           # [18, 64, 3, 3]
    wl = np.transpose(w_scaled, (2, 3, 1, 0)).reshape(K2, C, G * K2)
    wl32 = np.zeros((K2, C, 32), np.float32)
    wl32[:, :, : G * K2] = wl
    wts = np.ascontiguousarray(
        np.concatenate([wl32, wl32], axis=1).transpose(1, 0, 2)
    ).astype(ml_dtypes.bfloat16)
    # -> [128, 9, 32]

    bias = np.zeros((128, 1), np.float32)
    for q in range(4):
        bias[32 * q : 32 * q + G * K2, 0] = shift

    sel = np.zeros((128, 4), np.float32)
    for q in range(4):
        sel[32 * q : 32 * q + G * K2, q] = 1.0
    sel = sel.astype(ml_dtypes.bfloat16)

    # esel[:, 2k+c, :]: lhsT mapping F rows -> channel-layout partitions for
    # tap k, chunk c.
    esel = np.zeros((128, 2 * K2, 128), np.float32)
    for k in range(K2):
        for c in range(2):
            for j in range(128):
                h, g = j // 64, (j % 64) // 32
                esel[32 * (2 * h + c) + g * K2 + k, 2 * k + c, j] = 1.0
    esel = esel.astype(ml_dtypes.bfloat16)

    # rsel: broadcast r rows q -> compact-layout partitions (32q..32q+32)
    rsel = np.zeros((4, 128), np.float32)
    for p in range(128):
        rsel[p // 32, p] = 1.0

    ident = np.eye(128, dtype=np.float32).astype(ml_dtypes.bfloat16)

    xpad = np.pad(x, ((0, 0), (0, 0), (1, 1), (1, 1)), mode="reflect")

    in_maps = []
    for core in range(NCORES):
        n, h = core // 2, core % 2
        r0 = 64 * h
        slab_a = xpad[n, :, r0 : r0 + SLAB_R, :]
        slab_b = xpad[n, :, r0 + SLAB_R - 1 : r0 + 2 * SLAB_R - 1, :]
        xab = np.ascontiguousarray(
            np.concatenate([slab_a, slab_b], axis=0), np.float32
        )
        xcen = np.ascontiguousarray(xab[:, 1::2, 1:129:2], np.float32)
        in_maps.append(
            {"xab": xab.astype(ml_dtypes.bfloat16), "xcen": xcen,
             "wts": wts, "bias": bias, "sel": sel,
             "esel": esel, "rsel": rsel, "ident": ident}
        )
    return in_maps


def _gather_output(results):
    out = np.empty((N, C, HO, WO), np.float32)
    for core, res in enumerate(results):
        n, h = core // 2, core % 2
        ycore = res["y"].reshape(2, C, ROWS_SUB, WO)
        out[n, :, 32 * h : 32 * h + ROWS_SUB, :] = ycore[0]
        out[n, :, 32 * h + ROWS_SUB : 32 * h + 2 * ROWS_SUB, :] = ycore[1]
    return out


def _ensure_ntff_hook():
    """Install the axon NTFF profile hook if the image's antenv lacks it."""
    try:
        from antenv import axon_hooks  # noqa: F401
        return
    except ImportError:
        pass
    try:
        import sys
        import types

        import antenv
        from trn_agent_boot.trn_boot import _ntff_profile_via_ctypes

        hook = _ntff_profile_via_ctypes("/opt/axon/libaxon_pjrt.so")
        mod = types.ModuleType("antenv.axon_hooks")
        state = {"hook": hook}
        mod.get_axon_ntff_profile_hook = lambda: state["hook"]
        mod.set_axon_ntff_profile_hook = lambda h: state.update(hook=h)
        sys.modules["antenv.axon_hooks"] = mod
        antenv.axon_hooks = mod
    except Exception:
        pass


def kernel(x, conv_w, gamma, beta, running_mean, running_var):
    global _compiled
    x = np.asarray(x, np.float32)
    conv_w = np.asarray(conv_w, np.float32)
    gamma = np.asarray(gamma, np.float32)
    beta = np.asarray(beta, np.float32)
    running_mean = np.asarray(running_mean, np.float32)
    running_var = np.asarray(running_var, np.float32)

    if _compiled is None:
        _compiled = _build_program()
    nc = _compiled

    in_maps = _host_inputs(x, conv_w, gamma, beta, running_mean, running_var)
    trace = bool(int(os.environ.get("PASA_TRACE", "0")))
    if trace:
        _ensure_ntff_hook()
    res = run_bass_kernel_spmd(
        nc, in_maps, core_ids=list(range(NCORES)), trace=trace
    )
    kernel.last_results = res
    return _gather_output(res.results)


if __name__ == "__main__":
    # quick CoreSim check of core 0 against a numpy re-implementation
    from concourse.bass_interp import CoreSim

    rng = np.random.default_rng(0)
    x = rng.standard_normal((N, C, H, W), np.float32)
    conv_w = (rng.standard_normal((G * K2, C, K, K), np.float32)
              * np.sqrt(2.0 / (G * K2 * K * K)))
    gamma = rng.uniform(0.5, 1.5, G * K2).astype(np.float32)
    beta = (rng.standard_normal(G * K2) * 0.1).astype(np.float32)
    rmean = (rng.standard_normal(G * K2) * 0.1).astype(np.float32)
    rvar = rng.uniform(0.5, 1.5, G * K2).astype(np.float32)

    nc = _build_program()
    in_maps = _host_inputs(x, conv_w, gamma, beta, rmean, rvar)
    sim = CoreSim(nc)
    for k, v in in_maps[0].items():
        sim.tensor(k)[:] = v
    sim.simulate(check_with_hw=False)
    ysim = np.array(sim.tensor("y")).reshape(2, C, ROWS_SUB, WO)

    # numpy reference for core 0 region (image 0, output rows 0..32)
    scale = gamma / np.sqrt(rvar + EPS)
    shift = beta - rmean * scale
    xpad = np.pad(x[0], ((0, 0), (1, 1), (1, 1)), mode="reflect")
    sig = np.zeros((G * K2, 32, WO), np.float32)
    for o in range(G * K2):
        for dy in range(K):
            for dx in range(K):
                sig[o] += np.einsum(
                    "crw->rw",
                    conv_w[o, :, dy, dx][:, None, None]
                    * xpad[:, dy : dy + 64 : 2, dx : dx + 128 : 2],
                )
    sig = sig * scale[:, None, None] + shift[:, None, None]
    e = np.exp(sig)
    r = 1.0 / e.sum(0)
    acc = np.zeros((C, 32, WO), np.float32)
    for g in range(G):
        for k in range(K2):
            dy, dx = k // K, k % K
            acc[32 * g : 32 * g + 32] += (
                xpad[32 * g : 32 * g + 32, dy : dy + 64 : 2, dx : dx + 128 : 2]
                * e[g * K2 + k][None]
            )
    ref = (xpad[:, 1:65:2, 1:129:2] - acc * r[None]).astype(np.float32)

    got = np.concatenate([ysim[0], ysim[1]], axis=1)
    err = np.abs(got - ref).max() / np.abs(ref).max()
    print("sim rel err:", err)


# revision 20
# speedup vs baseline: 1.3000x; 1.0716x over previous
"""PASA group-softmax high-pass downsample kernel for 8 Trainium2 NeuronCores.

Reference computation (n=4, c=64, h=w=128, G=2 groups, K=3, stride 2):
  xp     = reflect_pad(x, 1)
  sigma  = conv3x3(xp, conv_w)                    # [n, 18, h, w]
  sigma  = sigma * bn_scale + bn_shift            # BN (inference)
  sigma  = softmax(sigma, axis=1)                 # over all 18 channels
  sigma  = onehot(center) - sigma                 # high-pass
  out[n,g,c,i,j] = sum_k patches[n,g,c,k,i,j] * sigma[n,g,k,i,j]
  return out[:, :, ::2, ::2]                      # [4, 64, 64, 64]

Key optimizations:
  - Only stride-2 output positions are computed (4x less conv/softmax/apply
    work than the reference, which computes all positions then subsamples).
  - BN is folded into the conv weights (host-side) + exp bias (on ACT).
  - softmax division is folded to the end: out = x_c - (sum_k xp_k*E_k) * r
    with r = 1/sum(E), so the per-tap multiplier is just E = exp(sigma_bn).
  - Sharding: core = (image n, h-half). Each core's region is further split
    into two sub-halves (A/B) stacked on SBUF partitions 64..127, so every
    vector op runs with all 128 lanes active.

Per-core device layout:
  x slab  [128 part, 33 rows, 130 cols]: part p<64 -> channel p, sub-half A
          (padded rows r0..r0+32); p>=64 -> channel p-64, sub-half B
          (padded rows r0+32..r0+64). Host prepares this (reflect pad+halo).
  conv    -> PSUM sigma [128, 512]: col-group q=(half, chunk) holds channels
          at partitions 32q..32q+32 (rows 18..31 zero), 512 positions each
          (chunk = 8 output rows x 64 cols).
  exp     -> E [128, 512] in SBUF (ACT, bias = BN shift per partition).
  D       -> ones-selector matmul -> [4, 512]; r = 1/D on DVE.
  Ebig_k  -> DMA row-broadcast of E rows into channel layout [128, 16, 64].
  apply   -> DVE: acc += patch_k * Ebig_k  (9 taps); out = x_c - acc*rbig.
"""

import os
import ml_dtypes
import numpy as np

import concourse.bass as bass
import concourse.tile as tile
from concourse import bacc, mybir
from concourse.bass_utils import run_bass_kernel_spmd

F32 = mybir.dt.float32
BF16 = mybir.dt.bfloat16

N, C, H, W = 4, 64, 128, 128
G, K = 2, 3
K2 = K * K
EPS = 1e-5
NCORES = 8
HO, WO = H // 2, W // 2            # 64, 64 output spatial
ROWS_PER_CORE = HO // 2            # 32 output rows per core (half image)
ROWS_SUB = ROWS_PER_CORE // 2      # 16 output rows per sub-half (A/B)
SLAB_R, SLAB_C = 2 * ROWS_SUB + 1, W + 2   # 33 x 130 padded slab per sub-half
POS_SUB = ROWS_SUB * WO            # 1024 positions per sub-half
CHUNK_ROWS = ROWS_SUB // 2         # 8 output rows per psum chunk
CHUNK = CHUNK_ROWS * WO            # 512 positions per chunk

_compiled = None


def _build_program():
    """Build the single SPMD Bass program (same for all 8 cores)."""
    nc = bacc.Bacc(
        "TRN2", target_bir_lowering=False, debug=False, num_devices=NCORES
    )

    xab = nc.dram_tensor("xab", [128, SLAB_R, SLAB_C], BF16, kind="ExternalInput")
    xcen = nc.dram_tensor("xcen", [128, ROWS_SUB, WO], F32, kind="ExternalInput")
    wts = nc.dram_tensor("wts", [128, K2, 32], BF16, kind="ExternalInput")
    bias = nc.dram_tensor("bias", [128, 1], F32, kind="ExternalInput")
    sel = nc.dram_tensor("sel", [128, 4], BF16, kind="ExternalInput")
    esel = nc.dram_tensor("esel", [128, 2 * K2, 128], BF16, kind="ExternalInput")
    rsel = nc.dram_tensor("rsel", [4, 128], F32, kind="ExternalInput")
    ident = nc.dram_tensor("ident", [128, 128], BF16, kind="ExternalInput")
    y = nc.dram_tensor("y", [128, ROWS_SUB, WO], F32, kind="ExternalOutput")
    warm_out = nc.dram_tensor("warm_out", [1, 2], F32, kind="ExternalOutput")

    with tile.TileContext(nc) as tc:
        with (
            tc.tile_pool(name="singles", bufs=1) as singles,
            tc.tile_pool(name="psum", bufs=1, space="PSUM") as psum,
            tc.tile_pool(name="ebig", bufs=2, space="PSUM") as ebig_pool,
            tc.tile_pool(name="work", bufs=3) as work,
        ):
            # ---- small loads first (sync ring), bulk x on gpsimd+scalar ----
            ident_sb = singles.tile([128, 128], BF16)
            nc.sync.dma_start(ident_sb[:], ident.ap())
            w_sb = singles.tile([128, K2, 32], BF16)
            nc.sync.dma_start(w_sb[:], wts.ap())
            bias_sb = singles.tile([128, 1], F32)
            nc.sync.dma_start(bias_sb[:], bias.ap())
            sel_sb = singles.tile([128, 4], BF16)
            nc.sync.dma_start(sel_sb[:], sel.ap())
            rsel_sb = singles.tile([4, 128], F32)
            nc.sync.dma_start(rsel_sb[:], rsel.ap())
            esel_sb = singles.tile([128, 2 * K2, 128], BF16)
            nc.sync.dma_start(esel_sb[:], esel.ap())
            xc_sb = singles.tile([128, ROWS_SUB, WO], F32)
            nc.sync.dma_start(xc_sb[:], xcen.ap())

            # prewarm ACT's exp table with a dep-free activate so the
            # table load doesn't land on the critical path (and doesn't
            # block the scalar DMA ring)
            warm_in = work.tile([1, 1], F32, tag="warm_in")
            nc.gpsimd.memset(warm_in[:], 0.25)
            warm_e = work.tile([1, 1], F32, tag="warm")
            nc.scalar.activation(warm_e[:], warm_in[:],
                                 mybir.ActivationFunctionType.Exp)

            # x slab: 4 row-chunk DMAs across two rings; A chunks land first
            x_sb = singles.tile([128, SLAB_R, SLAB_C], BF16)
            for h in range(2):
                p0 = 64 * h
                for ch in range(2):
                    r0 = 0 if ch == 0 else 2 * CHUNK_ROWS + 1
                    r1 = 2 * CHUNK_ROWS + 1 if ch == 0 else SLAB_R
                    eng = nc.gpsimd if ch == 0 else nc.scalar
                    eng.dma_start(
                        x_sb[p0 : p0 + 64, r0:r1],
                        xab.ap()[p0 : p0 + 64, r0:r1],
                    )

            # PE warm-up: junk matmuls on already-loaded constants keep the
            # HAM activity window busy during the x load so the conv runs
            # at 2.4 GHz from its first tap.  The accumulation chain plus the
            # warm_out reader keeps DCE from dropping them.
            warm_ps = psum.tile([128, 128], F32, tag="dr",
                                 padded_shape=[128, CHUNK])
            NWARM = 95
            for i in range(NWARM):
                nc.tensor.matmul(warm_ps[:], ident_sb[:], ident_sb[:],
                                 start=(i == 0), stop=(i == NWARM - 1),
                                 skip_group_check=True)
            warm_sb = work.tile([1, 2], F32, tag="warm_sb")
            nc.vector.tensor_copy(warm_sb[:], warm_ps[0:1, 0:2])
            nc.sync.dma_start(warm_out.ap(), warm_sb[:])

            # ---- conv: 9 taps x 4 col-groups into one PSUM bank ----
            sigma_ps = psum.tile([128, CHUNK], F32, tag="acc",
                                 padded_shape=[128, POS_SUB])
            for k in range(K2):
                dy, dx = k // K, k % K
                for q in range(4):
                    h, ch = q // 2, q % 2
                    p0 = 64 * h
                    r0 = 2 * CHUNK_ROWS * ch + dy
                    rhs = x_sb[
                        p0 : p0 + 64,
                        r0 : r0 + 2 * (CHUNK_ROWS - 1) + 1 : 2,
                        dx : dx + 2 * (WO - 1) + 1 : 2,
                    ]
                    nc.tensor.matmul(
                        sigma_ps[32 * q : 32 * q + 32, :],
                        w_sb[p0 : p0 + 64, k, :],
                        rhs,
                        start=(k == 0),
                        stop=(k == K2 - 1),
                        tile_position=(p0, 32 * q),
                        skip_group_check=True,
                    )

            # ---- E = exp(sigma + bn_shift), in bf16 ----
            e_sb = singles.tile([128, CHUNK], BF16)
            nc.scalar.activation(
                e_sb[:], sigma_ps[:], mybir.ActivationFunctionType.Exp,
                bias=bias_sb[:], scale=1.0,
            )

            # ---- denominator, reciprocal, normalized weights F = E/D ----
            d_ps = psum.tile([4, CHUNK], F32, tag="dr")
            nc.tensor.matmul(d_ps[:], sel_sb[:], e_sb[:])
            r_sb = singles.tile([4, CHUNK], F32)
            r_scr = singles.tile([4, CHUNK], F32)
            nc.vector.reciprocal_approx_accurate(r_sb[:], d_ps[:], r_scr[:])
            rbig18_ps = psum.tile([128, CHUNK], F32, tag="dr")
            nc.tensor.matmul(rbig18_ps[:], rsel_sb[:], r_sb[:])
            f_sb = singles.tile([128, CHUNK], BF16)
            nc.vector.tensor_mul(f_sb[:], e_sb[:], rbig18_ps[:])

            # ---- apply: acc_ps = sum_k patch_k * Fbig_k (adds on PE) ----
            acc_ps = psum.tile([128, POS_SUB], F32, tag="acc")
            prods = []
            for k in range(K2):
                dy, dx = k // K, k % K
                ebig = ebig_pool.tile([128, POS_SUB], F32, name=f"ebig{k}",
                                      tag="ebig")
                for ch in range(2):
                    nc.tensor.matmul(
                        ebig[:, CHUNK * ch : CHUNK * (ch + 1)],
                        esel_sb[:, 2 * k + ch, :],
                        f_sb[:],
                    )
                patch = x_sb[:, dy : dy + 2 * (ROWS_SUB - 1) + 1 : 2,
                             dx : dx + 2 * (WO - 1) + 1 : 2]
                prod = work.tile([128, POS_SUB], BF16, name=f"prod{k}",
                                 tag="prod")
                nc.vector.tensor_mul(
                    prod[:].rearrange("p (r c) -> p r c", r=ROWS_SUB),
                    patch, ebig[:].rearrange("p (r c) -> p r c", r=ROWS_SUB),
                )
                prods.append(prod)
                if k >= 1:
                    pprev = prods[k - 1]
                    for ch in range(2):
                        nc.tensor.matmul(
                            acc_ps[:, CHUNK * ch : CHUNK * (ch + 1)],
                            ident_sb[:],
                            pprev[:, CHUNK * ch : CHUNK * (ch + 1)],
                            start=(k == 1),
                            stop=False,
                            skip_group_check=True,
                        )
            for ch in range(2):
                nc.tensor.matmul(
                    acc_ps[:, CHUNK * ch : CHUNK * (ch + 1)],
                    ident_sb[:],
                    prods[K2 - 1][:, CHUNK * ch : CHUNK * (ch + 1)],
                    start=False,
                    stop=True,
                    skip_group_check=True,
                )

            # ---- combine: y = x_center - acc (2 chunks, stores overlap) ----
            y_sb = work.tile([128, ROWS_SUB, WO], F32)
            acc3 = acc_ps[:].rearrange("p (r c) -> p r c", r=ROWS_SUB)
            for ch in range(2):
                rr = slice(CHUNK_ROWS * ch, CHUNK_ROWS * (ch + 1))
                nc.vector.tensor_sub(y_sb[:, rr], xc_sb[:, rr],
                                     acc3[:, rr])
                eng = nc.sync if ch == 0 else nc.scalar
                eng.dma_start(y.ap()[:, rr], y_sb[:, rr])

    nc.compile()
    return nc


def _host_inputs(x, conv_w, gamma, beta, running_mean, running_var):
    """Prepare per-core input dicts (sharding + BN folding + reflect pad)."""
    scale = gamma / np.sqrt(running_var + EPS)
    shift = beta - running_mean * scale

    # weights: lhsT layout [tap, c, o] scaled by BN, padded to 32 outs, dup'd
    w_scaled = conv_w * scale[:, None, None, None]           # [18, 64, 3, 3]
    wl = np.transpose(w_scaled, (2, 3, 1, 0)).reshape(K2, C, G * K2)
    wl32 = np.zeros((K2, C, 32), np.float32)
    wl32[:, :, : G * K2] = wl
    wts = np.ascontiguousarray(
        np.concatenate([wl32, wl32], axis=1).transpose(1, 0, 2)
    ).astype(ml_dtypes.bfloat16)
    # -> [128, 9, 32]

    bias = np.zeros((128, 1), np.float32)
    for q in range(4):
        bias[32 * q : 32 * q + G * K2, 0] = shift

    sel = np.zeros((128, 4), np.float32)
    for q in range(4):
        sel[32 * q : 32 * q + G * K2, q] = 1.0
    sel = sel.astype(ml_dtypes.bfloat16)

    # esel[:, 2k+c, :]: lhsT mapping F rows -> channel-layout partitions for
    # tap k, chunk c.
    esel = np.zeros((128, 2 * K2, 128), np.float32)
    for k in range(K2):
        for c in range(2):
            for j in range(128):
                h, g = j // 64, (j % 64) // 32
                esel[32 * (2 * h + c) + g * K2 + k, 2 * k + c, j] = 1.0
    esel = esel.astype(ml_dtypes.bfloat16)

    # rsel: broadcast r rows q -> compact-layout partitions (32q..32q+32)
    rsel = np.zeros((4, 128), np.float32)
    for p in range(128):
        rsel[p // 32, p] = 1.0

    ident = np.eye(128, dtype=np.float32).astype(ml_dtypes.bfloat16)

    xpad = np.pad(x, ((0, 0), (0, 0), (1, 1), (1, 1)), mode="reflect")

    in_maps = []
    for core in range(NCORES):
        n, h = core // 2, core % 2
        r0 = 64 * h
        slab_a = xpad[n, :, r0 : r0 + SLAB_R, :]
        slab_b = xpad[n, :, r0 + SLAB_R - 1 : r0 + 2 * SLAB_R - 1, :]
        xab = np.ascontiguousarray(
            np.concatenate([slab_a, slab_b], axis=0), np.float32
        )
        xcen = np.ascontiguousarray(xab[:, 1::2, 1:129:2], np.float32)
        in_maps.append(
            {"xab": xab.astype(ml_dtypes.bfloat16), "xcen": xcen,
             "wts": wts, "bias": bias, "sel": sel,
             "esel": esel, "rsel": rsel, "ident": ident}
        )
    return in_maps


def _gather_output(results):
    out = np.empty((N, C, HO, WO), np.float32)
    for core, res in enumerate(results):
        n, h = core // 2, core % 2
        ycore = res["y"].reshape(2, C, ROWS_SUB, WO)
        out[n, :, 32 * h : 32 * h + ROWS_SUB, :] = ycore[0]
        out[n, :, 32 * h + ROWS_SUB : 32 * h + 2 * ROWS_SUB, :] = ycore[1]
    return out


def _ensure_ntff_hook():
    """Install the axon NTFF profile hook if the image's antenv lacks it."""
    try:
        from antenv import axon_hooks  # noqa: F401
        return
    except ImportError:
        pass
    try:
        import sys
        import types

        import antenv
        from trn_agent_boot.trn_boot import _ntff_profile_via_ctypes

        hook = _ntff_profile_via_ctypes("/opt/axon/libaxon_pjrt.so")
        mod = types.ModuleType("antenv.axon_hooks")
        state = {"hook": hook}
        mod.get_axon_ntff_profile_hook = lambda: state["hook"]
        mod.set_axon_ntff_profile_hook = lambda h: state.update(hook=h)
        sys.modules["antenv.axon_hooks"] = mod
        antenv.axon_hooks = mod
    except Exception:
        pass


def kernel(x, conv_w, gamma, beta, running_mean, running_var):
    global _compiled
    x = np.asarray(x, np.float32)
    conv_w = np.asarray(conv_w, np.float32)
    gamma = np.asarray(gamma, np.float32)
    beta = np.asarray(beta, np.float32)
    running_mean = np.asarray(running_mean, np.float32)
    running_var = np.asarray(running_var, np.float32)

    if _compiled is None:
        _compiled = _build_program()
    nc = _compiled

    in_maps = _host_inputs(x, conv_w, gamma, beta, running_mean, running_var)
    trace = bool(int(os.environ.get("PASA_TRACE", "0")))
    if trace:
        _ensure_ntff_hook()
    res = run_bass_kernel_spmd(
        nc, in_maps, core_ids=list(range(NCORES)), trace=trace
    )
    kernel.last_results = res
    return _gather_output(res.results)


if __name__ == "__main__":
    # quick CoreSim check of core 0 against a numpy re-implementation
    from concourse.bass_interp import CoreSim

    rng = np.random.default_rng(0)
    x = rng.standard_normal((N, C, H, W), np.float32)
    conv_w = (rng.standard_normal((G * K2, C, K, K), np.float32)
              * np.sqrt(2.0 / (G * K2 * K * K)))
    gamma = rng.uniform(0.5, 1.5, G * K2).astype(np.float32)
    beta = (rng.standard_normal(G * K2) * 0.1).astype(np.float32)
    rmean = (rng.standard_normal(G * K2) * 0.1).astype(np.float32)
    rvar = rng.uniform(0.5, 1.5, G * K2).astype(np.float32)

    nc = _build_program()
    in_maps = _host_inputs(x, conv_w, gamma, beta, rmean, rvar)
    sim = CoreSim(nc)
    for k, v in in_maps[0].items():
        sim.tensor(k)[:] = v
    sim.simulate(check_with_hw=False)
    ysim = np.array(sim.tensor("y")).reshape(2, C, ROWS_SUB, WO)

    # numpy reference for core 0 region (image 0, output rows 0..32)
    scale = gamma / np.sqrt(rvar + EPS)
    shift = beta - rmean * scale
    xpad = np.pad(x[0], ((0, 0), (1, 1), (1, 1)), mode="reflect")
    sig = np.zeros((G * K2, 32, WO), np.float32)
    for o in range(G * K2):
        for dy in range(K):
            for dx in range(K):
                sig[o] += np.einsum(
                    "crw->rw",
                    conv_w[o, :, dy, dx][:, None, None]
                    * xpad[:, dy : dy + 64 : 2, dx : dx + 128 : 2],
                )
    sig = sig * scale[:, None, None] + shift[:, None, None]
    e = np.exp(sig)
    r = 1.0 / e.sum(0)
    acc = np.zeros((C, 32, WO), np.float32)
    for g in range(G):
        for k in range(K2):
            dy, dx = k // K, k % K
            acc[32 * g : 32 * g + 32] += (
                xpad[32 * g : 32 * g + 32, dy : dy + 64 : 2, dx : dx + 128 : 2]
                * e[g * K2 + k][None]
            )
    ref = (xpad[:, 1:65:2, 1:129:2] - acc * r[None]).astype(np.float32)

    got = np.concatenate([ysim[0], ysim[1]], axis=1)
    err = np.abs(got - ref).max() / np.abs(ref).max()
    print("sim rel err:", err)
